# revision 1
# baseline (speedup 1.0000x reference)
"""Grouped SwiGLU MoE FFN (8 experts) on 8 Trainium2 NeuronCores.

Expert-parallel: core e owns expert e's weights and its contiguous slice of
tokens (inputs arrive pre-sorted by expert).  Per core we compute
    g = silu(x_e @ w1_e.T); u = x_e @ w3_e.T; y_e = (g*u) @ w2_e.T
with fp32r (1-pass FP22) matmuls on the PE array.

All matmul operands need the contraction dim on SBUF partitions, so the host
pre-packs x/w1/w3/w2 into partition-major tiled layouts (pure numpy
transposes) and un-packs the output.  Device kernel processes tokens in two
halves of 1024 to fit SBUF.
"""

import sys

sys.path.insert(0, "/opt/trn_rl_repo")

import numpy as np

import concourse.bass as bass
import concourse.mybir as mybir
import concourse.tile as tile
from concourse import bacc
from concourse.bass import ts
from concourse.bass_utils import run_bass_kernel_spmd

F32 = mybir.dt.float32
F32R = mybir.dt.float32r

E, H, D, T = 8, 1408, 2048, 16384
TE = T // E            # tokens per expert (uniform fast path)
TH = TE // 2           # half of tokens processed at a time
NT = TH // 512         # 512-wide t tiles per half
KD = D // 128          # contraction tiles over d
JH = H // 128          # contraction tiles over h / h strips
ID = D // 128          # output d strips


def _build_program():
    nc = bacc.Bacc("TRN2", target_bir_lowering=False, debug=False, num_devices=E)

    xt_d = nc.dram_tensor("xt", [2, 128, KD, TH], F32, kind="ExternalInput").ap()
    w13_d = nc.dram_tensor("w13", [JH, 2, 128, KD, 128], F32, kind="ExternalInput").ap()
    w2_d = nc.dram_tensor("w2t", [ID, 128, JH, 128], F32, kind="ExternalInput").ap()
    y_d = nc.dram_tensor("y", [2, ID, 128, TH], F32, kind="ExternalOutput").ap()

    with tile.TileContext(nc) as tc:
        with (
            tc.tile_pool(name="xp", bufs=1) as xp,
            tc.tile_pool(name="wp", bufs=3) as wp,
            tc.tile_pool(name="hp", bufs=1) as hp,
            tc.tile_pool(name="sp", bufs=2) as sp,
            tc.tile_pool(name="yp", bufs=2) as yp,
            tc.tile_pool(name="ps", bufs=2, space="PSUM") as ps,
        ):
            for hf in range(2):
                # First matmul needs only w13[j=0, s=0] and xt[k=0, t<512];
                # issue DMAs in exactly the order the j=0 matmul stream
                # consumes them so PE starts as early as possible.
                w13_next = wp.tile([128, 2, KD, 128], F32R, tag="w13", name="w13p")
                nc.sync.dma_start(w13_next[:, 0], w13_d[0, 0].bitcast(F32R))
                xt = xp.tile([128, KD, TH], F32R, tag="xt")
                for tt in range(NT):
                    nc.sync.dma_start(
                        xt[:, 0, ts(tt, 512)], xt_d[hf, :, 0, ts(tt, 512)].bitcast(F32R)
                    )
                nc.sync.dma_start(w13_next[:, 1], w13_d[0, 1].bitcast(F32R))
                for k in range(1, KD):
                    for tt in range(NT):
                        nc.sync.dma_start(
                            xt[:, k, ts(tt, 512)],
                            xt_d[hf, :, k, ts(tt, 512)].bitcast(F32R),
                        )

                hh = []
                for j in range(JH):
                    w13 = w13_next
                    if j + 1 < JH:
                        w13_next = wp.tile(
                            [128, 2, KD, 128], F32R, tag="w13", name="w13p"
                        )
                        nc.sync.dma_start(w13_next[:, 0], w13_d[j + 1, 0].bitcast(F32R))
                        nc.sync.dma_start(w13_next[:, 1], w13_d[j + 1, 1].bitcast(F32R))

                    hh_j = hp.tile([128, TH], F32R, tag=f"hh{j}")
                    pg = [ps.tile([128, 512], F32, tag="pg", name=f"pg{tt}") for tt in range(NT)]
                    pu = [ps.tile([128, 512], F32, tag="pu", name=f"pu{tt}") for tt in range(NT)]
                    for k in range(KD):
                        for tt in range(NT):
                            nc.tensor.matmul(
                                pg[tt][:], w13[:, 0, k, :], xt[:, k, ts(tt, 512)],
                                start=(k == 0), stop=(k == KD - 1),
                            )
                    for k in range(KD):
                        for tt in range(NT):
                            nc.tensor.matmul(
                                pu[tt][:], w13[:, 1, k, :], xt[:, k, ts(tt, 512)],
                                start=(k == 0), stop=(k == KD - 1),
                            )
                    for tt in range(NT):
                        sg = sp.tile([128, 512], F32, tag="sg")
                        nc.scalar.activation(
                            sg[:], pg[tt][:], mybir.ActivationFunctionType.Silu
                        )
                        nc.vector.tensor_mul(hh_j[:, ts(tt, 512)], sg[:], pu[tt][:])
                    hh.append(hh_j)

                for i in range(ID):
                    w2 = wp.tile([128, JH, 128], F32R, tag="w2")
                    nc.sync.dma_start(w2[:], w2_d[i].bitcast(F32R))
                    y_sb = yp.tile([128, TH], F32, tag="ysb")
                    for tt in range(NT):
                        py = ps.tile([128, 512], F32, tag="py")
                        for j in range(JH):
                            nc.tensor.matmul(
                                py[:], w2[:, j, :], hh[j][:, ts(tt, 512)],
                                start=(j == 0), stop=(j == JH - 1),
                            )
                        nc.vector.tensor_copy(y_sb[:, ts(tt, 512)], py[:])
                        nc.sync.dma_start(
                            y_d[hf, i, :, ts(tt, 512)], y_sb[:, ts(tt, 512)]
                        )

    nc.compile()
    return nc


_NC = None


def _get_nc():
    global _NC
    if _NC is None:
        _NC = _build_program()
    return _NC


def _prep_core_inputs(x_e, w1_e, w3_e, w2_e):
    # xt[hf, p, k, t] = x_e[hf*TH + t, k*128 + p]
    xt = np.empty((2, 128, KD, TH), dtype=np.float32)
    for hf in range(2):
        xh = x_e[hf * TH:(hf + 1) * TH].T           # [D, TH]
        xt[hf] = xh.reshape(KD, 128, TH).transpose(1, 0, 2)
    # w13[j, s, p, k, h] = w{1,3}_e[j*128 + h, k*128 + p]
    w1r = w1_e.reshape(JH, 128, KD, 128).transpose(0, 3, 2, 1)
    w3r = w3_e.reshape(JH, 128, KD, 128).transpose(0, 3, 2, 1)
    w13 = np.ascontiguousarray(np.stack([w1r, w3r], axis=1))
    # w2t[i, p, j, dd] = w2_e[i*128 + dd, j*128 + p]
    w2t = np.ascontiguousarray(w2_e.reshape(ID, 128, JH, 128).transpose(0, 3, 2, 1))
    return {
        "xt": np.ascontiguousarray(xt),
        "w13": w13,
        "w2t": w2t,
    }


def _reference_fallback(w1, w2, w3, x, counts):
    # Exact numpy mirror of the jax reference (incl. scatter-drop / gather-clamp)
    e, h, d = w1.shape
    t = x.shape[0]
    cap = 2 * (t // e)
    counts = counts.astype(np.int64)
    offsets = np.concatenate([[0], np.cumsum(counts)[:-1]])
    eid = np.repeat(np.arange(e), counts)[:t]
    pos = np.arange(t) - offsets[eid]
    buf = np.zeros((e, cap, d), np.float32)
    ok = pos < cap
    buf[eid[ok], pos[ok]] = x[ok]
    out = np.empty((e, cap, d), np.float32)
    for ee in range(e):
        a = buf[ee] @ w1[ee].T
        g = a / (1.0 + np.exp(-a))
        u = buf[ee] @ w3[ee].T
        out[ee] = (g * u) @ w2[ee].T
    pos_c = np.minimum(pos, cap - 1)
    return out[eid, pos_c]


def kernel(w1, w2, w3, x, num_tokens_per_expert):
    w1 = np.asarray(w1, dtype=np.float32)
    w2 = np.asarray(w2, dtype=np.float32)
    w3 = np.asarray(w3, dtype=np.float32)
    x = np.asarray(x, dtype=np.float32)
    counts = np.asarray(num_tokens_per_expert).astype(np.int32)

    if not (x.shape == (T, D) and w1.shape == (E, H, D)
            and np.all(counts == TE)):
        return _reference_fallback(w1, w2, w3, x, counts)

    nc = _get_nc()
    in_maps = []
    for e in range(E):
        in_maps.append(
            _prep_core_inputs(x[e * TE:(e + 1) * TE], w1[e], w3[e], w2[e])
        )
    res = run_bass_kernel_spmd(nc, in_maps, list(range(E)))

    out = np.empty((T, D), dtype=np.float32)
    for e in range(E):
        y = res.results[e]["y"]  # [2, ID, 128, TH]
        for hf in range(2):
            out[e * TE + hf * TH: e * TE + (hf + 1) * TH] = (
                y[hf].reshape(D, TH).T
            )
    return out



# revision 3
# speedup vs baseline: 1.3027x; 1.3027x over previous
"""Grouped SwiGLU MoE FFN (8 experts) on 8 Trainium2 NeuronCores.

Expert-parallel: core e owns expert e's weights and its contiguous slice of
tokens (inputs arrive pre-sorted by expert).  Per core we compute
    g = silu(x_e @ w1_e.T); u = x_e @ w3_e.T; y_e = (g*u) @ w2_e.T

Matmuls run on the PE array as fp8(e4m3) DoubleRow pairs at 0.5 cycles/row,
2x the bf16/fp32r rate.  To stay inside the 2e-2 error budget each operand
is split into hi+lo e4m3 parts (a "Karatsuba" split): for y = a@b we compute
a_hi@b_hi + a_lo@b_hi + a_hi@b_lo and drop the lo@lo term, giving ~bf16
accuracy at 0.75x the bf16 PE cost.  Each DoubleRow instruction packs two
independent 128-deep products: hi@hi terms pair adjacent contraction strips
(k, k+1); the two cross terms for one strip share one instruction via
stationary slot order (lo,hi) against moving slot order (hi,lo).

Weights are pre-scaled by 2^8 on the host so their hi parts sit in e4m3's
normal range; the scale is folded back in on the Act engine (silu input
scale and the final psum->bf16 copy).  All hi/lo splitting and layout
packing for x/w1/w3/w2 happens on the host in numpy; the device sees fp8
operands laid out partition-major and streams:
  phase A: per h-strip j, per 512-token tile: g,u psums -> silu (ACT),
           h = g*u (DVE) -> h_hi, h_lo fp8 tiles (DVE)
  phase B: per d-strip i: y psum over 12 padded h-strips -> bf16 out.
"""

import sys

sys.path.insert(0, "/opt/trn_rl_repo")

import numpy as np
import ml_dtypes

import concourse.bass as bass
import concourse.mybir as mybir
import concourse.tile as tile
from concourse import bacc
from concourse.bass import ts
from concourse.bass_utils import run_bass_kernel_spmd

F32 = mybir.dt.float32
BF16 = mybir.dt.bfloat16
F8 = mybir.dt.float8e4
NP_F8 = ml_dtypes.float8_e4m3fn
DR = mybir.MatmulPerfMode.DoubleRow
MULT = mybir.AluOpType.mult
SUBTRACT = mybir.AluOpType.subtract

E, H, D, T = 8, 1408, 2048, 16384
TE = T // E            # tokens per expert (uniform fast path)
KD = D // 128          # 16 contraction strips over d
JH = H // 128          # 11 h strips
JH2 = JH + 1           # padded to even for DoubleRow hi@hi pairing in mm2
ID = D // 128          # 16 output d strips
NT = TE // 512         # 4 token tiles of 512
WS = 256.0             # weight pre-scale (2^8) for e4m3 range health


def _build_program():
    nc = bacc.Bacc("TRN2", target_bir_lowering=False, debug=False, num_devices=E)

    # [p, k, sl, t]: sl 0=hi, 1=lo of x[t, 128k+p]
    x_d = nc.dram_tensor("xq", [128, KD, 2, TE], F8, kind="ExternalInput").ap()
    # [p, j, s, k, l, m]: s 0=w1,1=w3; l 0=lo,1=hi of (WS*w)[128j+m, 128k+p]
    w13_d = nc.dram_tensor("w13q", [128, JH, 2, KD, 2, 128], F8,
                           kind="ExternalInput").ap()
    # [p, i, kk, l, m]: l 0=lo,1=hi of (WS*w2)[128i+m, 128kk+p]; kk=11 zero pad
    w2_d = nc.dram_tensor("w2q", [128, ID, JH2, 2, 128], F8,
                          kind="ExternalInput").ap()
    # [i, p, t] = y[t, 128i+p]
    y_d = nc.dram_tensor("y", [ID, 128, TE], BF16, kind="ExternalOutput").ap()

    with tile.TileContext(nc) as tc:
        with (
            tc.tile_pool(name="xp", bufs=1) as xp,
            tc.tile_pool(name="wp", bufs=3) as wp,
            tc.tile_pool(name="w2p", bufs=5) as w2p,
            tc.tile_pool(name="hp", bufs=1) as hp,
            tc.tile_pool(name="sp", bufs=2) as sp,
            tc.tile_pool(name="fp", bufs=2) as fp,
            tc.tile_pool(name="yp", bufs=2) as yp,
            tc.tile_pool(name="psA", bufs=2, space="PSUM") as psA,
            tc.tile_pool(name="psB", bufs=3, space="PSUM") as psB,
        ):
            # h strips in fp8 hi/lo; strip JH (=11) is the zero pad for mm2
            # hi@hi pairing (its w2 slot is also zero, but memset keeps any
            # stale NaN encodings out of the pair).
            h = hp.tile([128, JH2, 2, TE], F8, tag="h")
            nc.vector.memset(h[:, JH, 0, :], 0.0)

            w13_cur = wp.tile([128, 2, KD, 2, 128], F8, tag="w13", name="w13t")
            nc.sync.dma_start(w13_cur[:], w13_d[:, 0])
            xt = xp.tile([128, KD, 2, TE], F8, tag="xt")
            nc.sync.dma_start(xt[:, :, :, ts(0, 512)], x_d[:, :, :, ts(0, 512)])
            w13_nxt = wp.tile([128, 2, KD, 2, 128], F8, tag="w13", name="w13t")
            nc.sync.dma_start(w13_nxt[:], w13_d[:, 1])
            for tt in range(1, NT):
                nc.sync.dma_start(
                    xt[:, :, :, ts(tt, 512)], x_d[:, :, :, ts(tt, 512)]
                )
            # w2 prefetch (behind x/w13 in queue order; needed only in phase B)
            w2_tiles = []
            for i in range(4):
                w2t = w2p.tile([128, JH2, 2, 128], F8, tag="w2", name="w2t")
                nc.sync.dma_start(w2t[:], w2_d[:, i])
                w2_tiles.append(w2t)

            # ---- phase A: g/u matmuls + h build ----
            for j in range(JH):
                w13 = w13_cur
                w13_cur = w13_nxt
                if j + 2 < JH:
                    w13_nxt = wp.tile(
                        [128, 2, KD, 2, 128], F8, tag="w13", name="w13t"
                    )
                    nc.sync.dma_start(w13_nxt[:], w13_d[:, j + 2])
                for tt in range(NT):
                    tsl = ts(tt, 512)
                    pg = psA.tile([128, 512], F32, tag="pg")
                    pu = psA.tile([128, 512], F32, tag="pu")
                    for s, ps_ in ((0, pg), (1, pu)):
                        # hi@hi over strip pairs (k, k+1)
                        for k in range(0, KD, 2):
                            nc.tensor.matmul(
                                ps_[:], w13[:, s, k:k + 2, 1, :],
                                xt[:, k:k + 2, 0, tsl],
                                start=(k == 0), stop=False, perf_mode=DR,
                            )
                        # cross terms: stationary (lo,hi) x moving (hi,lo)
                        for k in range(KD):
                            nc.tensor.matmul(
                                ps_[:], w13[:, s, k, :, :], xt[:, k, :, tsl],
                                start=False, stop=(k == KD - 1), perf_mode=DR,
                            )
                    sg = sp.tile([128, 512], F32, tag="sg")
                    nc.scalar.activation(
                        sg[:], pg[:], mybir.ActivationFunctionType.Silu,
                        scale=1.0 / WS,
                    )
                    hf = fp.tile([128, 512], F32, tag="hf")
                    # hf = (pu * 1/WS) * sg = u * g
                    nc.vector.scalar_tensor_tensor(
                        hf[:], pu[:], 1.0 / WS, sg[:], op0=MULT, op1=MULT
                    )
                    nc.vector.tensor_copy(h[:, j, 0, tsl], hf[:])
                    nc.vector.tensor_sub(h[:, j, 1, tsl], hf[:], h[:, j, 0, tsl])

            # ---- phase B: y matmuls + store ----
            for i in range(ID):
                if i + 4 < ID:
                    w2t = w2p.tile([128, JH2, 2, 128], F8, tag="w2", name="w2t")
                    nc.sync.dma_start(w2t[:], w2_d[:, i + 4])
                    w2_tiles.append(w2t)
                w2 = w2_tiles[i]
                y_sb = yp.tile([128, TE], BF16, tag="ysb")
                for tt in range(NT):
                    tsl = ts(tt, 512)
                    py = psB.tile([128, 512], F32, tag="py")
                    for kk in range(0, JH2, 2):
                        nc.tensor.matmul(
                            py[:], w2[:, kk:kk + 2, 1, :],
                            h[:, kk:kk + 2, 0, tsl],
                            start=(kk == 0), stop=False, perf_mode=DR,
                        )
                    for kk in range(JH):
                        nc.tensor.matmul(
                            py[:], w2[:, kk, :, :], h[:, kk, :, tsl],
                            start=False, stop=(kk == JH - 1), perf_mode=DR,
                        )
                    nc.scalar.activation(
                        y_sb[:, tsl], py[:],
                        mybir.ActivationFunctionType.Copy, scale=1.0 / WS,
                    )
                nc.sync.dma_start(y_d[i], y_sb[:])

    nc.compile()
    return nc


_NC = None


def _get_nc():
    global _NC
    if _NC is None:
        _NC = _build_program()
    return _NC


def _hilo(a):
    hi = a.astype(NP_F8)
    lo = (a - hi.astype(np.float32)).astype(NP_F8)
    return hi, lo


def _prep_core_inputs(x_e, w1_e, w3_e, w2_e):
    # xq[p, k, sl, t] with sl=(hi,lo) of x_e[t, 128k+p]
    xr = np.ascontiguousarray(x_e.T).reshape(KD, 128, TE)   # [k, p, t]
    x_hi, x_lo = _hilo(xr)
    xq = np.stack([x_hi, x_lo], axis=2)                      # [k, p, sl, t]
    xq = np.ascontiguousarray(xq.transpose(1, 0, 2, 3))      # [p, k, sl, t]

    # w13q[p, j, s, k, l, m] with l=(lo,hi) of WS*w[128j+m, 128k+p]
    def pack_w13(w):
        r = (w * WS).reshape(JH, 128, KD, 128)               # [j, m, k, p]
        hi, lo = _hilo(r)
        t = np.stack([lo, hi], axis=3)                       # [j, m, k, l, p]
        return t.transpose(4, 0, 2, 3, 1)                    # [p, j, k, l, m]

    w13q = np.stack([pack_w13(w1_e), pack_w13(w3_e)], axis=2)  # [p,j,s,k,l,m]
    w13q = np.ascontiguousarray(w13q)

    # w2q[p, i, kk, l, m] with l=(lo,hi) of WS*w2[128i+m, 128kk+p]; pad kk=11
    r2 = (w2_e * WS).reshape(ID, 128, JH, 128)               # [i, m, kk, p]
    hi2, lo2 = _hilo(r2)
    t2 = np.stack([lo2, hi2], axis=3)                        # [i, m, kk, l, p]
    t2 = t2.transpose(4, 0, 2, 3, 1)                         # [p, i, kk, l, m]
    w2q = np.zeros((128, ID, JH2, 2, 128), dtype=NP_F8)
    w2q[:, :, :JH] = t2
    return {"xq": xq, "w13q": w13q, "w2q": np.ascontiguousarray(w2q)}


def _reference_fallback(w1, w2, w3, x, counts):
    # Exact numpy mirror of the jax reference (incl. scatter-drop / gather-clamp)
    e, h, d = w1.shape
    t = x.shape[0]
    cap = 2 * (t // e)
    counts = counts.astype(np.int64)
    offsets = np.concatenate([[0], np.cumsum(counts)[:-1]])
    eid = np.repeat(np.arange(e), counts)[:t]
    pos = np.arange(t) - offsets[eid]
    buf = np.zeros((e, cap, d), np.float32)
    ok = pos < cap
    buf[eid[ok], pos[ok]] = x[ok]
    out = np.empty((e, cap, d), np.float32)
    for ee in range(e):
        a = buf[ee] @ w1[ee].T
        g = a / (1.0 + np.exp(-a))
        u = buf[ee] @ w3[ee].T
        out[ee] = (g * u) @ w2[ee].T
    pos_c = np.minimum(pos, cap - 1)
    return out[eid, pos_c]


def kernel(w1, w2, w3, x, num_tokens_per_expert):
    w1 = np.asarray(w1, dtype=np.float32)
    w2 = np.asarray(w2, dtype=np.float32)
    w3 = np.asarray(w3, dtype=np.float32)
    x = np.asarray(x, dtype=np.float32)
    counts = np.asarray(num_tokens_per_expert).astype(np.int32)

    if not (x.shape == (T, D) and w1.shape == (E, H, D)
            and np.all(counts == TE)):
        return _reference_fallback(w1, w2, w3, x, counts)

    nc = _get_nc()
    in_maps = []
    for e in range(E):
        in_maps.append(
            _prep_core_inputs(x[e * TE:(e + 1) * TE], w1[e], w3[e], w2[e])
        )
    res = run_bass_kernel_spmd(nc, in_maps, list(range(E)))

    out = np.empty((T, D), dtype=np.float32)
    for e in range(E):
        y = res.results[e]["y"]  # [ID, 128, TE] bf16
        out[e * TE:(e + 1) * TE] = (
            y.astype(np.float32).transpose(2, 0, 1).reshape(TE, D)
        )
    return out
